# revision 10
# baseline (speedup 1.0000x reference)
# Multi-head attention with per-head relative position embeddings
# (music-transformer style masked rel-to-abs skew), on 8 TRN2 NeuronCores.
#
# Sharding: core c handles batch b = c//2 and head group g = c%2 (4 heads).
# Each core computes its heads' attention plus a partial output projection
# (wo columns for its head group); host sums the two partials per batch.
#
# The rel-to-abs skew (pad/reshape/slice in the reference) is done exactly
# via a DRAM bounce: R = q @ rel_k^T tiles are written row-major, then read
# back with an affine access pattern whose row stride is one element short
# (L-1 / L against an L / L+1 row pitch), which reproduces the reference's
# flattened-padded indexing including the wrap-around "garbage" region.
# When the host detects a causal mask it builds a band-only variant that
# skips fully-masked work; otherwise an exact general-mask variant runs.

from contextlib import ExitStack

import numpy as np

B, L, D, H = 4, 1024, 512, 8
DK = D // H          # 64
M = 1024
HPC = H // 2         # heads per core = 4
GD = HPC * DK        # head-group width = 256
NCORES = 8
NEG = -1.0e9

_prog_cache = {}


def _build(causal: bool):
    import concourse.bass as bass
    import concourse.mybir as mybir
    import concourse.tile as tile
    from concourse import bacc

    f32 = mybir.dt.float32
    fp16 = mybir.dt.float16
    bf16 = mybir.dt.bfloat16
    i32 = mybir.dt.int32
    Act = mybir.ActivationFunctionType
    Alu = mybir.AluOpType

    nc = bacc.Bacc("TRN2")

    # ---- I/O ----
    xqT = nc.dram_tensor("xqT", [D, L], f32, kind="ExternalInput")
    xkT = nc.dram_tensor("xkT", [D, L], f32, kind="ExternalInput")
    xvT = nc.dram_tensor("xvT", [D, L], f32, kind="ExternalInput")
    wqT = nc.dram_tensor("wqT", [D, GD], f32, kind="ExternalInput")
    wkT = nc.dram_tensor("wkT", [D, GD], f32, kind="ExternalInput")
    wvT = nc.dram_tensor("wvT", [D, GD], f32, kind="ExternalInput")
    woT = nc.dram_tensor("woT", [GD, D], f32, kind="ExternalInput")
    bqs = nc.dram_tensor("bqs", [GD, 1], f32, kind="ExternalInput")
    bks = nc.dram_tensor("bks", [GD, 1], f32, kind="ExternalInput")
    bvs = nc.dram_tensor("bvs", [1, GD], f32, kind="ExternalInput")
    bob = nc.dram_tensor("bob", [1, D], f32, kind="ExternalInput")
    relkT = nc.dram_tensor("relkT", [HPC, DK, M], f32, kind="ExternalInput")
    tpdT = nc.dram_tensor("tpdT", [DK, M], f32, kind="ExternalInput")
    ppdT = nc.dram_tensor("ppdT", [DK, M], f32, kind="ExternalInput")
    eyeb = nc.dram_tensor("eyeb", [128, 128], fp16, kind="ExternalInput")
    if causal:
        trilb = nc.dram_tensor("trilb", [128, 128], f32, kind="ExternalInput")
    else:
        maskin = nc.dram_tensor("maskin", [L, L], i32, kind="ExternalInput")
    out = nc.dram_tensor("out", [L, D], f32, kind="ExternalOutput")

    PW = M if causal else (M + 1)  # bounce row pitch

    with tile.TileContext(nc) as tc, ExitStack() as ctx:
        pers = ctx.enter_context(tc.tile_pool(name="pers", bufs=1))
        ppool = ctx.enter_context(tc.tile_pool(name="ps", bufs=1, space="PSUM"))
        pR = ctx.enter_context(tc.tile_pool(name="pR", bufs=3))
        pB = ctx.enter_context(tc.tile_pool(name="pB", bufs=3))
        pL = ctx.enter_context(tc.tile_pool(name="pL", bufs=3))
        pA = ctx.enter_context(tc.tile_pool(name="pA", bufs=3))
        pAT = ctx.enter_context(tc.tile_pool(name="pAT", bufs=3))
        pS = ctx.enter_context(tc.tile_pool(name="pS", bufs=8))
        pO = ctx.enter_context(tc.tile_pool(name="pO", bufs=2))
        pM = ctx.enter_context(tc.tile_pool(name="pM", bufs=2))
        dpool = ctx.enter_context(tc.tile_pool(name="dram", bufs=1, space="DRAM"))

        bounce = dpool.tile([HPC, L, PW], fp16)

        # ---- persistent loads (f32 staged, converted to fp16 on gpsimd) ----
        def load16(name, shape, src):
            st = pM.tile(shape, f32, tag="stage")
            nc.sync.dma_start(out=st, in_=src)
            dst = pers.tile(shape, fp16, tag=name)
            nc.gpsimd.tensor_copy(dst, st)
            return dst

        sq = load16("sq", [128, 4, L], xqT.rearrange("(t p) l -> p t l", p=128))
        sk = load16("sk", [128, 4, L], xkT.rearrange("(t p) l -> p t l", p=128))
        sv = load16("sv", [128, 4, L], xvT.rearrange("(t p) l -> p t l", p=128))
        swq = load16("swq", [128, 4, GD], wqT.rearrange("(t p) g -> p t g", p=128))
        swk = load16("swk", [128, 4, GD], wkT.rearrange("(t p) g -> p t g", p=128))
        swv = load16("swv", [128, 4, GD], wvT.rearrange("(t p) g -> p t g", p=128))
        swo = load16("swo", [128, 2, D], woT.rearrange("(t p) d -> p t d", p=128))
        sbq = pers.tile([128, 2, 1], f32)
        sbk = pers.tile([128, 2, 1], f32)
        nc.sync.dma_start(out=sbq, in_=bqs.rearrange("(t p) o -> p t o", p=128))
        nc.sync.dma_start(out=sbk, in_=bks.rearrange("(t p) o -> p t o", p=128))
        sbv = pers.tile([128, GD], f32)
        sbo = pers.tile([128, D], f32)
        nc.sync.dma_start(out=sbv, in_=bvs[:, :].broadcast_to((128, GD)))
        nc.sync.dma_start(out=sbo, in_=bob[:, :].broadcast_to((128, D)))
        srel_st = pM.tile([128, HPC, M], f32, tag="stage")
        nc.sync.dma_start(out=srel_st[0:DK], in_=relkT.rearrange("h d m -> d h m"))
        nc.sync.dma_start(out=srel_st[DK:128], in_=relkT.rearrange("h d m -> d h m"))
        srel = pers.tile([128, HPC, M], fp16)
        nc.gpsimd.tensor_copy(srel, srel_st)
        stpd = pers.tile([128, M], f32)
        sppd = pers.tile([128, M], f32)
        nc.sync.dma_start(out=stpd[0:DK], in_=tpdT[:, :])
        nc.sync.dma_start(out=stpd[DK:128], in_=tpdT[:, :])
        nc.sync.dma_start(out=sppd[0:DK], in_=ppdT[:, :])
        nc.sync.dma_start(out=sppd[DK:128], in_=ppdT[:, :])
        seyeb = pers.tile([128, 128], fp16)
        nc.sync.dma_start(out=seyeb, in_=eyeb[:, :])
        if causal:
            stril = pers.tile([128, 128], f32)
            nc.sync.dma_start(out=stril, in_=trilb[:, :])
        else:
            smaskb = pers.tile([128, 8, L], bf16)
            for it in range(8):
                mt = pM.tile([128, L], i32)
                nc.sync.dma_start(out=mt, in_=maskin[it * 128:(it + 1) * 128, :])
                # {0,1} -> {-1e9, 0}
                nc.scalar.activation(out=smaskb[:, it, :], in_=mt,
                                     func=Act.Identity, bias=NEG, scale=-NEG)

        # ---- projections ----
        # time+pitch diagonal folds into khT's psum (same term for every head)
        stp = pers.tile([128, M], f32)
        nc.vector.tensor_add(stp, stpd, sppd)
        # qhT/khT: [dk_local(2x128 part-tiles), L]  = w_s @ x^T + b
        qhT = pers.tile([128, 2, L], fp16)
        khT = pers.tile([128, 2, L], fp16)
        for dst, w, b in ((qhT, swq, sbq), (khT, swk, sbk)):
            for t in range(2):
                for lc in range(2):
                    ps = ppool.tile([128, 512], f32, tag="mm1")
                    for dc in range(4):
                        nc.tensor.matmul(
                            ps,
                            lhsT=w[:, dc, t * 128:(t + 1) * 128],
                            rhs=sq[:, dc, lc * 512:(lc + 1) * 512] if dst is qhT
                            else sk[:, dc, lc * 512:(lc + 1) * 512],
                            start=(dc == 0), stop=(dc == 3),
                        )
                    if dst is khT:
                        nc.vector.tensor_add(ps, ps,
                                             stp[:, lc * 512:(lc + 1) * 512])
                    nc.scalar.activation(out=dst[:, t, lc * 512:(lc + 1) * 512],
                                         in_=ps, func=Act.Identity,
                                         bias=b[:, t, :], scale=1.0)

        # vh: [L(8x128), dk_local 256] bf16, = x @ w_s.T + b
        vh = pers.tile([128, 8, GD], fp16)
        for lt in range(8):
            ps = ppool.tile([128, GD], f32, tag="mm1")
            for dc in range(4):
                nc.tensor.matmul(
                    ps,
                    lhsT=sv[:, dc, lt * 128:(lt + 1) * 128],
                    rhs=swv[:, dc, :],
                    start=(dc == 0), stop=(dc == 3),
                )
            nc.vector.tensor_add(vh[:, lt, :], ps, sbv)

        # ---- attention units ----
        ohT = pers.tile([128, 2, L], fp16)  # [dk_local, L] attention out (transposed)
        for hh in range(HPC):
            t, po = hh // 2, (hh % 2) * DK
            for it in range(8):
                i0 = it * 128
                cols = i0 + 128 if causal else L
                nblk = cols // 128
                ncj = (cols + 511) // 512
                qsl = qhT[po:po + DK, t, i0:i0 + 128]

                # R = q_h @ rel_k^T  -> bounce (full rows; garbage region
                # of the affine re-read touches low columns too)
                psR = ppool.tile([128, M], f32, tag="R")
                for c in range(2):
                    nc.tensor.matmul(psR[:, c * 512:(c + 1) * 512],
                                     lhsT=qsl,
                                     rhs=srel[po:po + DK, hh, c * 512:(c + 1) * 512],
                                     start=True, stop=True)
                if causal:
                    rbw = pR.tile([128, M], fp16)
                    nc.scalar.copy(rbw, psR)
                    nc.sync.dma_start(out=bounce[hh, i0:i0 + 128, :], in_=rbw)
                else:
                    rbw = pR.tile([128, PW], fp16)
                    nc.vector.memset(rbw[:, 0:1], 0.0)
                    nc.scalar.copy(rbw[:, 1:], psR)
                    nc.sync.dma_start(out=bounce[hh, i0:i0 + 128, :], in_=rbw)

                # logits = q@k^T (+tp fold) + rel(skewed) + mask
                psL = ppool.tile([128, M], f32, tag="L")
                for c in range(ncj):
                    c1 = min(cols, (c + 1) * 512)
                    nc.tensor.matmul(psL[:, c * 512:c1],
                                     lhsT=qsl,
                                     rhs=khT[po:po + DK, t, c * 512:c1],
                                     start=True, stop=True)

                # skewed re-read of R: row stride PW-1 against pitch PW
                rb = pB.tile([128, M], fp16)
                bap = bounce[hh]
                nc.sync.dma_start(
                    out=rb[:, :cols],
                    in_=bass.AP(tensor=bap.tensor,
                                offset=bap.offset + i0 * (PW - 1) + (PW - 1),
                                ap=[[PW - 1, 128], [1, cols]]),
                )
                lg = pL.tile([128, M], f32)
                nc.vector.tensor_add(lg[:, :cols], psL[:, :cols], rb[:, :cols])
                if causal:
                    dsl = lg[:, i0:i0 + 128]
                    nc.vector.tensor_add(dsl, dsl, stril)
                else:
                    nc.vector.tensor_add(lg, lg, smaskb[:, it, :])

                # softmax (denominator via exp's accumulator)
                nm = pS.tile([128, 1], f32)
                nc.vector.reduce_max(nm, lg[:, :cols],
                                     axis=mybir.AxisListType.X, negate=True)
                A = pA.tile([128, M], fp16)
                rs = pS.tile([128, 1], f32)
                nc.scalar.activation(out=A[:, :cols], in_=lg[:, :cols],
                                     func=Act.Exp, bias=nm, scale=1.0,
                                     accum_out=rs)
                rinv = pS.tile([128, 1], f32)
                nc.vector.reciprocal(rinv, rs)
                nc.vector.tensor_scalar_mul(A[:, :cols], A[:, :cols], rinv)

                # A^T blocks then o^T = vh^T @ A^T (accumulate over j)
                psA = ppool.tile([128, 8, 128], fp16, tag="AT")
                for c in range(nblk):
                    nc.tensor.transpose(psA[:, c, :], A[:, c * 128:(c + 1) * 128],
                                        seyeb)
                AT = pAT.tile([128, 8, 128], fp16)
                nc.vector.tensor_copy(AT[:, :nblk, :], psA[:, :nblk, :])
                pso = ppool.tile([128, 128], f32, tag="o")
                for c in range(nblk):
                    nc.tensor.matmul(pso[po:po + DK, :],
                                     lhsT=vh[:, c, hh * DK:(hh + 1) * DK],
                                     rhs=AT[:, c, :],
                                     start=(c == 0), stop=(c == nblk - 1))
                nc.vector.tensor_copy(ohT[po:po + DK, t, i0:i0 + 128],
                                      pso[po:po + DK, :])

        # ---- output projection (partial over this head group) ----
        for lt in range(8):
            ps = ppool.tile([128, D], f32, tag="mm1")
            for t in range(2):
                nc.tensor.matmul(ps,
                                 lhsT=ohT[:, t, lt * 128:(lt + 1) * 128],
                                 rhs=swo[:, t, :],
                                 start=(t == 0), stop=(t == 1))
            so = pO.tile([128, D], f32)
            nc.vector.tensor_add(so, ps, sbo)
            nc.sync.dma_start(out=out[lt * 128:(lt + 1) * 128, :], in_=so)

    nc.compile()
    return nc


def _shards(inputs):
    q = np.asarray(inputs["q"], np.float32)
    k = np.asarray(inputs["k"], np.float32)
    v = np.asarray(inputs["v"], np.float32)
    mask = np.asarray(inputs["mask"])
    wq, bq = np.asarray(inputs["wq"], np.float32), np.asarray(inputs["bq"], np.float32)
    wk, bk = np.asarray(inputs["wk"], np.float32), np.asarray(inputs["bk"], np.float32)
    wv, bv = np.asarray(inputs["wv"], np.float32), np.asarray(inputs["bv"], np.float32)
    wo, bo = np.asarray(inputs["wo"], np.float32), np.asarray(inputs["bo"], np.float32)
    rel_k = np.asarray(inputs["rel_k"], np.float32)
    rel_time = np.asarray(inputs["rel_time"], np.float32)
    rel_pitch = np.asarray(inputs["rel_pitch"], np.float32)

    causal = bool(
        np.array_equal(mask[0], np.tril(np.ones((L, L), mask.dtype)))
        and all(np.array_equal(mask[b_], mask[0]) for b_ in range(1, B))
    )

    idx = np.arange(M)
    tpd = rel_time[idx, idx, :]    # [M, DK] diagonal (gather only, no math)
    ppd = rel_pitch[idx, idx, :]

    eyef = np.eye(128, dtype=np.float32)
    eyeb = np.eye(128, dtype=np.float16)
    if causal:
        r = np.arange(128)
        trilb = np.where(r[None, :] <= r[:, None], 0.0, NEG).astype(np.float32)

    in_maps = []
    for c in range(NCORES):
        b_, g = c // 2, c % 2
        rows = slice(g * GD, (g + 1) * GD)
        heads = slice(g * HPC, (g + 1) * HPC)
        m = {
            "xqT": np.ascontiguousarray(q[b_].T),
            "xkT": np.ascontiguousarray(k[b_].T),
            "xvT": np.ascontiguousarray(v[b_].T),
            "wqT": np.ascontiguousarray(wq[rows].T),
            "wkT": np.ascontiguousarray(wk[rows].T),
            "wvT": np.ascontiguousarray(wv[rows].T),
            "woT": np.ascontiguousarray(wo[:, rows].T),
            "bqs": np.ascontiguousarray(bq[rows].reshape(GD, 1)),
            "bks": np.ascontiguousarray(bk[rows].reshape(GD, 1)),
            "bvs": np.ascontiguousarray(bv[rows].reshape(1, GD)),
            "bob": np.ascontiguousarray(
                (bo if g == 0 else np.zeros_like(bo)).reshape(1, D)),
            "relkT": np.ascontiguousarray(rel_k[heads].transpose(0, 2, 1)),
            "tpdT": np.ascontiguousarray(tpd.T),
            "ppdT": np.ascontiguousarray(ppd.T),
            "eyeb": eyeb,
        }
        if causal:
            m["trilb"] = trilb
        else:
            m["maskin"] = np.ascontiguousarray(mask[b_].astype(np.int32))
        in_maps.append(m)
    return causal, in_maps


def kernel(**inputs) -> np.ndarray:
    from concourse.bass_utils import run_bass_kernel_spmd

    causal, in_maps = _shards(inputs)
    key = ("causal" if causal else "general",)
    if key not in _prog_cache:
        _prog_cache[key] = _build(causal)
    nc = _prog_cache[key]

    res = run_bass_kernel_spmd(nc, in_maps, core_ids=list(range(NCORES)))
    outs = [r["out"] for r in res.results]
    full = np.empty((B, L, D), np.float32)
    for b_ in range(B):
        full[b_] = outs[2 * b_] + outs[2 * b_ + 1]
    return full


# revision 13
# speedup vs baseline: 1.1953x; 1.1953x over previous
# Multi-head attention with per-head relative position embeddings
# (music-transformer style masked rel-to-abs skew), on 8 TRN2 NeuronCores.
#
# Sharding: core c handles batch b = c//2 and head group g = c%2 (4 heads).
# Each core computes its heads' attention plus a partial output projection
# (wo columns for its head group); host sums the two partials per batch.
#
# The rel-to-abs skew (pad/reshape/slice in the reference) is done exactly
# via a DRAM bounce: R = q @ rel_k^T tiles are written row-major, then read
# back with an affine access pattern whose row stride is one element short
# (L-1 / L against an L / L+1 row pitch), which reproduces the reference's
# flattened-padded indexing including the wrap-around "garbage" region.
# When the host detects a causal mask it builds a band-only variant that
# skips fully-masked work; otherwise an exact general-mask variant runs.

from contextlib import ExitStack

import numpy as np

B, L, D, H = 4, 1024, 512, 8
DK = D // H          # 64
M = 1024
HPC = H // 2         # heads per core = 4
GD = HPC * DK        # head-group width = 256
NCORES = 8
NEG = -1.0e9

_prog_cache = {}


def _build(causal: bool):
    import concourse.bass as bass
    import concourse.mybir as mybir
    import concourse.tile as tile
    from concourse import bacc

    f32 = mybir.dt.float32
    fp16 = mybir.dt.float16
    bf16 = mybir.dt.bfloat16
    i32 = mybir.dt.int32
    Act = mybir.ActivationFunctionType
    Alu = mybir.AluOpType

    nc = bacc.Bacc("TRN2")

    # ---- I/O ----
    xqT = nc.dram_tensor("xqT", [D, L], fp16, kind="ExternalInput")
    xkT = nc.dram_tensor("xkT", [D, L], fp16, kind="ExternalInput")
    xvT = nc.dram_tensor("xvT", [D, L], fp16, kind="ExternalInput")
    wqT = nc.dram_tensor("wqT", [D, GD], fp16, kind="ExternalInput")
    wkT = nc.dram_tensor("wkT", [D, GD], fp16, kind="ExternalInput")
    wvT = nc.dram_tensor("wvT", [D, GD], fp16, kind="ExternalInput")
    woT = nc.dram_tensor("woT", [GD, D], fp16, kind="ExternalInput")
    bqs = nc.dram_tensor("bqs", [GD, 1], f32, kind="ExternalInput")
    bks = nc.dram_tensor("bks", [GD, 1], f32, kind="ExternalInput")
    bvs = nc.dram_tensor("bvs", [1, GD], f32, kind="ExternalInput")
    bob = nc.dram_tensor("bob", [1, D], f32, kind="ExternalInput")
    relkT = nc.dram_tensor("relkT", [HPC, DK, M], fp16, kind="ExternalInput")
    tpdT = nc.dram_tensor("tpdT", [DK, M], f32, kind="ExternalInput")
    ppdT = nc.dram_tensor("ppdT", [DK, M], f32, kind="ExternalInput")
    eyeb = nc.dram_tensor("eyeb", [128, 128], fp16, kind="ExternalInput")
    if causal:
        trilb = nc.dram_tensor("trilb", [128, 128], f32, kind="ExternalInput")
    else:
        maskin = nc.dram_tensor("maskin", [L, L], i32, kind="ExternalInput")
    out = nc.dram_tensor("out", [L, D], f32, kind="ExternalOutput")

    PW = M if causal else (M + 1)  # bounce row pitch

    with tile.TileContext(nc) as tc, ExitStack() as ctx:
        pers = ctx.enter_context(tc.tile_pool(name="pers", bufs=1))
        ppool = ctx.enter_context(tc.tile_pool(name="ps", bufs=1, space="PSUM"))
        pR = ctx.enter_context(tc.tile_pool(name="pR", bufs=3))
        pB = ctx.enter_context(tc.tile_pool(name="pB", bufs=3))
        pL = ctx.enter_context(tc.tile_pool(name="pL", bufs=3))
        pA = ctx.enter_context(tc.tile_pool(name="pA", bufs=3))
        pAT = ctx.enter_context(tc.tile_pool(name="pAT", bufs=3))
        pS = ctx.enter_context(tc.tile_pool(name="pS", bufs=8))
        pO = ctx.enter_context(tc.tile_pool(name="pO", bufs=2))
        pM = ctx.enter_context(tc.tile_pool(name="pM", bufs=2))
        dpool = ctx.enter_context(tc.tile_pool(name="dram", bufs=1, space="DRAM"))

        bounce = dpool.tile([HPC, L, PW], fp16)

        # ---- persistent loads (fp16 shipped from host) ----
        def load(name, shape, src):
            dst = pers.tile(shape, fp16, tag=name)
            nc.sync.dma_start(out=dst, in_=src)
            return dst

        sq = load("sq", [128, 4, L], xqT.rearrange("(t p) l -> p t l", p=128))
        sk = load("sk", [128, 4, L], xkT.rearrange("(t p) l -> p t l", p=128))
        sv = load("sv", [128, 4, L], xvT.rearrange("(t p) l -> p t l", p=128))
        swq = load("swq", [128, 4, GD], wqT.rearrange("(t p) g -> p t g", p=128))
        swk = load("swk", [128, 4, GD], wkT.rearrange("(t p) g -> p t g", p=128))
        swv = load("swv", [128, 4, GD], wvT.rearrange("(t p) g -> p t g", p=128))
        swo = load("swo", [128, 2, D], woT.rearrange("(t p) d -> p t d", p=128))
        sbq = pers.tile([128, 2, 1], f32)
        sbk = pers.tile([128, 2, 1], f32)
        nc.sync.dma_start(out=sbq, in_=bqs.rearrange("(t p) o -> p t o", p=128))
        nc.sync.dma_start(out=sbk, in_=bks.rearrange("(t p) o -> p t o", p=128))
        sbv = pers.tile([128, GD], f32)
        sbo = pers.tile([128, D], f32)
        nc.sync.dma_start(out=sbv, in_=bvs[:, :].broadcast_to((128, GD)))
        nc.sync.dma_start(out=sbo, in_=bob[:, :].broadcast_to((128, D)))
        srel = pers.tile([128, HPC, M], fp16)
        nc.sync.dma_start(out=srel[0:DK], in_=relkT.rearrange("h d m -> d h m"))
        nc.sync.dma_start(out=srel[DK:128], in_=relkT.rearrange("h d m -> d h m"))
        stpd = pers.tile([128, M], f32)
        sppd = pers.tile([128, M], f32)
        nc.sync.dma_start(out=stpd[0:DK], in_=tpdT[:, :])
        nc.sync.dma_start(out=stpd[DK:128], in_=tpdT[:, :])
        nc.sync.dma_start(out=sppd[0:DK], in_=ppdT[:, :])
        nc.sync.dma_start(out=sppd[DK:128], in_=ppdT[:, :])
        seyeb = pers.tile([128, 128], fp16)
        nc.sync.dma_start(out=seyeb, in_=eyeb[:, :])
        if causal:
            stril = pers.tile([128, 128], f32)
            nc.sync.dma_start(out=stril, in_=trilb[:, :])
        else:
            smaskb = pers.tile([128, 8, L], bf16)
            for it in range(8):
                mt = pM.tile([128, L], i32)
                nc.sync.dma_start(out=mt, in_=maskin[it * 128:(it + 1) * 128, :])
                # {0,1} -> {-1e9, 0}
                nc.scalar.activation(out=smaskb[:, it, :], in_=mt,
                                     func=Act.Identity, bias=NEG, scale=-NEG)

        # ---- projections ----
        # time+pitch diagonal folds into khT's psum (same term for every head)
        stp = pers.tile([128, M], f32)
        nc.vector.tensor_add(stp, stpd, sppd)
        # qhT/khT: [dk_local(2x128 part-tiles), L]  = w_s @ x^T + b
        qhT = pers.tile([128, 2, L], fp16)
        khT = pers.tile([128, 2, L], fp16)
        for dst, w, b in ((qhT, swq, sbq), (khT, swk, sbk)):
            for t in range(2):
                for lc in range(2):
                    ps = ppool.tile([128, 512], f32, tag="o")
                    for dc in range(4):
                        nc.tensor.matmul(
                            ps,
                            lhsT=w[:, dc, t * 128:(t + 1) * 128],
                            rhs=sq[:, dc, lc * 512:(lc + 1) * 512] if dst is qhT
                            else sk[:, dc, lc * 512:(lc + 1) * 512],
                            start=(dc == 0), stop=(dc == 3),
                        )
                    if dst is khT:
                        nc.vector.tensor_add(ps, ps,
                                             stp[:, lc * 512:(lc + 1) * 512])
                    nc.scalar.activation(out=dst[:, t, lc * 512:(lc + 1) * 512],
                                         in_=ps, func=Act.Identity,
                                         bias=b[:, t, :], scale=1.0)

        # vh: [L(8x128), dk_local 256] bf16, = x @ w_s.T + b
        vh = pers.tile([128, 8, GD], fp16)
        for lt in range(8):
            ps = ppool.tile([128, GD], f32, tag="o")
            for dc in range(4):
                nc.tensor.matmul(
                    ps,
                    lhsT=sv[:, dc, lt * 128:(lt + 1) * 128],
                    rhs=swv[:, dc, :],
                    start=(dc == 0), stop=(dc == 3),
                )
            nc.vector.tensor_add(vh[:, lt, :], ps, sbv)

        # ---- attention units ----
        ohT = pers.tile([128, 2, L], fp16)  # [dk_local, L] attention out (transposed)
        for hh in range(HPC):
            t, po = hh // 2, (hh % 2) * DK
            # R = q_h @ rel_k^T for all 8 row tiles -> one batched DMA to
            # the DRAM bounce (full rows; the affine re-read's garbage
            # region touches low columns too)
            rfull = pR.tile([128, 8, PW], fp16)
            for it in range(8):
                i0 = it * 128
                qsl = qhT[po:po + DK, t, i0:i0 + 128]
                psR = ppool.tile([128, M], f32, tag="R")
                for c in range(2):
                    nc.tensor.matmul(psR[:, c * 512:(c + 1) * 512],
                                     lhsT=qsl,
                                     rhs=srel[po:po + DK, hh, c * 512:(c + 1) * 512],
                                     start=True, stop=True)
                if causal:
                    nc.scalar.copy(rfull[:, it, :], psR)
                else:
                    nc.vector.memset(rfull[:, it, 0:1], 0.0)
                    nc.scalar.copy(rfull[:, it, 1:], psR)
            nc.sync.dma_start(
                out=bounce[hh].rearrange("(i p) w -> p i w", p=128),
                in_=rfull)

            for it in range(8):
                i0 = it * 128
                cols = i0 + 128 if causal else L
                nblk = cols // 128
                ncj = (cols + 511) // 512
                qsl = qhT[po:po + DK, t, i0:i0 + 128]

                # logits = q@k^T (+tp fold) + rel(skewed) + mask
                psL = ppool.tile([128, M], f32, tag="L", bufs=2)
                for c in range(ncj):
                    c1 = min(cols, (c + 1) * 512)
                    nc.tensor.matmul(psL[:, c * 512:c1],
                                     lhsT=qsl,
                                     rhs=khT[po:po + DK, t, c * 512:c1],
                                     start=True, stop=True)

                # skewed re-read of R: row stride PW-1 against pitch PW
                rb = pB.tile([128, M], fp16)
                bap = bounce[hh]
                nc.gpsimd.dma_start(
                    out=rb[:, :cols],
                    in_=bass.AP(tensor=bap.tensor,
                                offset=bap.offset + i0 * (PW - 1) + (PW - 1),
                                ap=[[PW - 1, 128], [1, cols]]),
                )
                lg = pL.tile([128, M], f32)
                nc.vector.tensor_add(lg[:, :cols], psL[:, :cols], rb[:, :cols])
                if causal:
                    dsl = lg[:, i0:i0 + 128]
                    nc.vector.tensor_add(dsl, dsl, stril)
                else:
                    nc.vector.tensor_add(lg, lg, smaskb[:, it, :])

                # softmax (denominator via exp's accumulator)
                nm = pS.tile([128, 1], f32)
                nc.vector.reduce_max(nm, lg[:, :cols],
                                     axis=mybir.AxisListType.X, negate=True)
                A = pA.tile([128, M], fp16)
                rs = pS.tile([128, 1], f32)
                nc.scalar.activation(out=A[:, :cols], in_=lg[:, :cols],
                                     func=Act.Exp, bias=nm, scale=1.0,
                                     accum_out=rs)
                rinv = pS.tile([128, 1], f32)
                nc.vector.reciprocal(rinv, rs)
                nc.vector.tensor_scalar_mul(A[:, :cols], A[:, :cols], rinv)

                # A^T blocks then o^T = vh^T @ A^T (accumulate over j)
                psA = ppool.tile([128, 8, 128], fp16, tag="AT")
                for c in range(nblk):
                    nc.tensor.transpose(psA[:, c, :], A[:, c * 128:(c + 1) * 128],
                                        seyeb)
                AT = pAT.tile([128, 8, 128], fp16)
                nc.vector.tensor_copy(AT[:, :nblk, :], psA[:, :nblk, :])
                pso = ppool.tile([128, 128], f32, tag="o")
                for c in range(nblk):
                    nc.tensor.matmul(pso[po:po + DK, :],
                                     lhsT=vh[:, c, hh * DK:(hh + 1) * DK],
                                     rhs=AT[:, c, :],
                                     start=(c == 0), stop=(c == nblk - 1))
                nc.vector.tensor_copy(ohT[po:po + DK, t, i0:i0 + 128],
                                      pso[po:po + DK, :])

        # ---- output projection (partial over this head group) ----
        for lt in range(8):
            ps = ppool.tile([128, D], f32, tag="o")
            for t in range(2):
                nc.tensor.matmul(ps,
                                 lhsT=ohT[:, t, lt * 128:(lt + 1) * 128],
                                 rhs=swo[:, t, :],
                                 start=(t == 0), stop=(t == 1))
            so = pO.tile([128, D], f32)
            nc.vector.tensor_add(so, ps, sbo)
            nc.sync.dma_start(out=out[lt * 128:(lt + 1) * 128, :], in_=so)

    nc.compile()
    return nc


def _shards(inputs):
    q = np.asarray(inputs["q"], np.float32)
    k = np.asarray(inputs["k"], np.float32)
    v = np.asarray(inputs["v"], np.float32)
    mask = np.asarray(inputs["mask"])
    wq, bq = np.asarray(inputs["wq"], np.float32), np.asarray(inputs["bq"], np.float32)
    wk, bk = np.asarray(inputs["wk"], np.float32), np.asarray(inputs["bk"], np.float32)
    wv, bv = np.asarray(inputs["wv"], np.float32), np.asarray(inputs["bv"], np.float32)
    wo, bo = np.asarray(inputs["wo"], np.float32), np.asarray(inputs["bo"], np.float32)
    rel_k = np.asarray(inputs["rel_k"], np.float32)
    rel_time = np.asarray(inputs["rel_time"], np.float32)
    rel_pitch = np.asarray(inputs["rel_pitch"], np.float32)

    causal = bool(
        np.array_equal(mask[0], np.tril(np.ones((L, L), mask.dtype)))
        and all(np.array_equal(mask[b_], mask[0]) for b_ in range(1, B))
    )

    idx = np.arange(M)
    tpd = rel_time[idx, idx, :]    # [M, DK] diagonal (gather only, no math)
    ppd = rel_pitch[idx, idx, :]

    eyef = np.eye(128, dtype=np.float32)
    eyeb = np.eye(128, dtype=np.float16)
    if causal:
        r = np.arange(128)
        trilb = np.where(r[None, :] <= r[:, None], 0.0, NEG).astype(np.float32)

    in_maps = []
    for c in range(NCORES):
        b_, g = c // 2, c % 2
        rows = slice(g * GD, (g + 1) * GD)
        heads = slice(g * HPC, (g + 1) * HPC)
        m = {
            "xqT": np.ascontiguousarray(q[b_].T.astype(np.float16)),
            "xkT": np.ascontiguousarray(k[b_].T.astype(np.float16)),
            "xvT": np.ascontiguousarray(v[b_].T.astype(np.float16)),
            "wqT": np.ascontiguousarray(wq[rows].T.astype(np.float16)),
            "wkT": np.ascontiguousarray(wk[rows].T.astype(np.float16)),
            "wvT": np.ascontiguousarray(wv[rows].T.astype(np.float16)),
            "woT": np.ascontiguousarray(wo[:, rows].T.astype(np.float16)),
            "bqs": np.ascontiguousarray(bq[rows].reshape(GD, 1)),
            "bks": np.ascontiguousarray(bk[rows].reshape(GD, 1)),
            "bvs": np.ascontiguousarray(bv[rows].reshape(1, GD)),
            "bob": np.ascontiguousarray(
                (bo if g == 0 else np.zeros_like(bo)).reshape(1, D)),
            "relkT": np.ascontiguousarray(rel_k[heads].transpose(0, 2, 1).astype(np.float16)),
            "tpdT": np.ascontiguousarray(tpd.T),
            "ppdT": np.ascontiguousarray(ppd.T),
            "eyeb": eyeb,
        }
        if causal:
            m["trilb"] = trilb
        else:
            m["maskin"] = np.ascontiguousarray(mask[b_].astype(np.int32))
        in_maps.append(m)
    return causal, in_maps


def kernel(**inputs) -> np.ndarray:
    from concourse.bass_utils import run_bass_kernel_spmd

    causal, in_maps = _shards(inputs)
    key = ("causal" if causal else "general",)
    if key not in _prog_cache:
        _prog_cache[key] = _build(causal)
    nc = _prog_cache[key]

    res = run_bass_kernel_spmd(nc, in_maps, core_ids=list(range(NCORES)))
    outs = [r["out"] for r in res.results]
    full = np.empty((B, L, D), np.float32)
    for b_ in range(B):
        full[b_] = outs[2 * b_] + outs[2 * b_ + 1]
    return full
